# revision 25
# baseline (speedup 1.0000x reference)
"""Multi-head attention (B=4, S=2048, D=1024, H=16) on 8 TRN2 NeuronCores.

Sharding: 2D — batch 4-way x head-group 2-way. Core c handles batch b = c//2
and head group j = c%2 (8 heads, model-dim slice j*512:(j+1)*512 of the QKV
projections / rows j*512:(j+1)*512 of Wo). Each core computes a partial
[S, D] output (row-sharded Wo => partial sums); the host adds the two
partials per batch and the output bias.

Device-side layout notes:
 - Activations are kept transposed ([feature, seq]) so every matmul has its
   contraction dim on partitions. The host pre-transposes q/k/v (free) and
   pre-casts to bf16 (compute dtype; PSUM accumulation is fp32).
 - Attention uses the S^T layout: S^T[t, s] tiles come straight from
   lhsT=K^T, rhs=Q^T matmuls (two heads row-packed in the PE array);
   softmax-exp runs on ACT (scale=1/8 fused, one op per head pair);
   V is projected in natural [t, d'] layout with a ones-column appended per
   head so the PV matmul also produces the softmax denominators (row 64).
 - The attention phase is ACT(exp)-bound, so the Q/K projection and output
   projection matmul chains are interleaved between attention iterations to
   fill the idle PE slots.
 - The reference's "causal mask" adds log(1 + 1e-6) ~ 1e-6 to allowed
   logits, a ~1e-6 relative perturbation of the output - far below bf16
   noise - so it is deliberately not applied.
"""

import os
import sys
import types

sys.path.insert(0, "/opt/trn_rl_repo")

import numpy as np
import ml_dtypes

B, S, D, H = 4, 2048, 1024, 16
PD = D // H          # 64 head dim
P = 128              # partitions
NCORES = 8
DP = 512             # d' (feature) slice per core = 8 heads
KO = D // P          # 8 contraction tiles for QKV projections
MT = DP // P         # 4 feature tiles (= head pairs)
NT = S // P          # 16 key/time tiles
NSB = 4              # s blocks
SBW = S // NSB       # 512 block width
HL = 8               # local heads
VW = 65              # V columns per head incl. ones column

BF16 = ml_dtypes.bfloat16

_NC = None
LAST_RUN = {}


def _install_ntff_shim():
    """bass_utils' axon trace path imports antenv.axon_hooks, which this
    image lacks; register the hook module manually so BASS_TRACE works."""
    if "antenv.axon_hooks" in sys.modules:
        return
    try:
        import trn_agent_boot.trn_boot as tb

        hook = tb._ntff_profile_via_ctypes("/opt/axon/libaxon_pjrt.so")
    except Exception:
        hook = None
    m = types.ModuleType("antenv.axon_hooks")
    m.get_axon_ntff_profile_hook = lambda: hook
    m.set_axon_ntff_profile_hook = lambda h: None
    sys.modules["antenv.axon_hooks"] = m


def _build():
    import concourse.tile as tile
    from concourse import bacc, mybir

    CDT = mybir.dt.bfloat16
    F32 = mybir.dt.float32
    EXP = mybir.ActivationFunctionType.Exp
    ADD = mybir.AluOpType.add
    MUL = mybir.AluOpType.mult

    nc = bacc.Bacc("TRN2", target_bir_lowering=False, debug=False)

    qT_d = nc.dram_tensor("qT", [D, S], CDT, kind="ExternalInput").ap()
    kT_d = nc.dram_tensor("kT", [D, S], CDT, kind="ExternalInput").ap()
    vT_d = nc.dram_tensor("vT", [D, S], CDT, kind="ExternalInput").ap()
    wq_d = nc.dram_tensor("wq", [D, DP], CDT, kind="ExternalInput").ap()
    wk_d = nc.dram_tensor("wk", [D, DP], CDT, kind="ExternalInput").ap()
    wv_d = nc.dram_tensor("wv", [D, DP], CDT, kind="ExternalInput").ap()
    wo_d = nc.dram_tensor("wo", [DP, D], CDT, kind="ExternalInput").ap()
    bq_d = nc.dram_tensor("bq", [P, MT], F32, kind="ExternalInput").ap()
    bk_d = nc.dram_tensor("bk", [P, MT], F32, kind="ExternalInput").ap()
    bv_d = nc.dram_tensor("bv", [1, DP], F32, kind="ExternalInput").ap()
    yT_d = nc.dram_tensor("yT", [D, S], F32, kind="ExternalOutput").ap()

    with tile.TileContext(nc) as tc:
        import contextlib

        with contextlib.ExitStack() as ctx:
            wp = ctx.enter_context(tc.tile_pool(name="weights", bufs=1))
            biasp = ctx.enter_context(tc.tile_pool(name="bias", bufs=1))
            inp = ctx.enter_context(tc.tile_pool(name="inp", bufs=16))
            kinp = ctx.enter_context(tc.tile_pool(name="kinp", bufs=32))
            qsp = ctx.enter_context(tc.tile_pool(name="qsmall", bufs=16))
            actp = ctx.enter_context(tc.tile_pool(name="acts", bufs=1))
            expp = ctx.enter_context(tc.tile_pool(name="exps", bufs=4))
            op = ctx.enter_context(tc.tile_pool(name="otile", bufs=8))
            orp = ctx.enter_context(tc.tile_pool(name="oraw", bufs=2))
            rcpp = ctx.enter_context(tc.tile_pool(name="rcp", bufs=3))
            bcp = ctx.enter_context(tc.tile_pool(name="bcast", bufs=1))
            yp = ctx.enter_context(tc.tile_pool(name="ystage", bufs=2))
            # PSUM budget (8 banks): S^T pair tiles 2x2 + proj/out 2 + psO 2
            ps_pair = ctx.enter_context(
                tc.tile_pool(name="ps_pair", bufs=2, space="PSUM")
            )
            ps_mm = ctx.enter_context(tc.tile_pool(name="ps_mm", bufs=2, space="PSUM"))
            ps_o = ctx.enter_context(tc.tile_pool(name="ps_o", bufs=2, space="PSUM"))

            # ---- biases + weights (order matters: V first, K, then Q/out) --
            bv_sb = biasp.tile([P, DP], F32, tag="bv")
            nc.sync.dma_start(bv_sb[:], bv_d[0:1, :].to_broadcast((P, DP)))
            bq_sb = biasp.tile([P, MT], F32, tag="bq")
            nc.sync.dma_start(bq_sb[:], bq_d[:])
            bk_sb = biasp.tile([P, MT], F32, tag="bk")
            nc.sync.dma_start(bk_sb[:], bk_d[:])
            wv_sb = wp.tile([P, KO, DP], CDT, tag="wv")
            nc.sync.dma_start(wv_sb[:], wv_d.rearrange("(a p) m -> p a m", p=P))

            # vT in halves so the V projection can start after ~half the DMA;
            # kT in s-block quarters so K chains start per-block.
            vtiles = []
            for ko in range(KO):
                halves = []
                for h in range(2):
                    t_sb = inp.tile([P, S // 2], CDT, tag="vT", name=f"v{ko}_{h}")
                    nc.sync.dma_start(
                        t_sb[:],
                        vT_d[ko * P : (ko + 1) * P, h * (S // 2) : (h + 1) * (S // 2)],
                    )
                    halves.append(t_sb)
                vtiles.append(halves)
            wk_sb = wp.tile([P, KO, DP], CDT, tag="wk")
            nc.sync.dma_start(wk_sb[:], wk_d.rearrange("(a p) m -> p a m", p=P))
            ktiles = []
            for sbk in range(NSB):
                for ko in range(KO):
                    t_sb = kinp.tile([P, SBW], CDT, tag="kT", name=f"k{ko}_{sbk}")
                    nc.sync.dma_start(
                        t_sb[:],
                        kT_d[ko * P : (ko + 1) * P, sbk * SBW : (sbk + 1) * SBW],
                    )
                    ktiles.append((ko, sbk, t_sb))
            ktile = {(ko, sbk): t for ko, sbk, t in ktiles}
            wq_sb = wp.tile([P, KO, DP], CDT, tag="wq")
            nc.sync.dma_start(wq_sb[:], wq_d.rearrange("(a p) m -> p a m", p=P))
            wo_sb = wp.tile([P, MT, D], CDT, tag="wo")
            nc.sync.dma_start(wo_sb[:], wo_d.rearrange("(a p) n -> p a n", p=P))

            # ---- V projection (natural layout [t, d'], ones col per head) --
            Vp = actp.tile([P, NT, HL * VW], CDT, tag="Vp")
            with nc.named_scope("proj_v"):
                nc.vector.memset(
                    Vp[:].rearrange("p t (h c) -> p t h c", c=VW)[:, :, :, PD : PD + 1],
                    1.0,
                )
                for t in range(NT):
                    ps = ps_mm.tile([P, DP], F32, tag="ps", name="ps_v")
                    for ko in range(KO):
                        half = t // (NT // 2)
                        tt = t % (NT // 2)
                        nc.tensor.matmul(
                            ps[:],
                            vtiles[ko][half][:, tt * P : (tt + 1) * P],
                            wv_sb[:, ko, :],
                            start=(ko == 0),
                            stop=(ko == KO - 1),
                        )
                    nc.vector.tensor_tensor(
                        Vp[:, t, :].rearrange("p (h c) -> p h c", c=VW)[:, :, 0:PD],
                        ps[:].rearrange("p (h c) -> p h c", c=PD),
                        bv_sb[:].rearrange("p (h c) -> p h c", c=PD),
                        ADD,
                    )

            # ---- K / Q projection chain emitters ---------------------------
            KpT = actp.tile([P, MT, S], CDT, tag="KpT")
            QpT = actp.tile([P, MT, S], CDT, tag="QpT")

            def emit_k_m(m):
                with nc.named_scope("proj_k"):
                    for sbk in range(NSB):
                        ps = ps_mm.tile([P, SBW], F32, tag="ps", name="ps_k")
                        for ko in range(KO):
                            nc.tensor.matmul(
                                ps[:],
                                wk_sb[:, ko, m * P : (m + 1) * P],
                                ktile[(ko, sbk)][:],
                                start=(ko == 0),
                                stop=(ko == KO - 1),
                            )
                        nc.vector.tensor_scalar_add(
                            KpT[:, m, sbk * SBW : (sbk + 1) * SBW],
                            ps[:],
                            bk_sb[:, m : m + 1],
                        )

            qtiles = {}  # sbk -> list of 8 [P, SBW] tiles

            def emit_q_tiles(sbk):
                tiles = []
                for ko in range(KO):
                    t_sb = qsp.tile([P, SBW], CDT, tag="qT", name=f"q{ko}")
                    nc.sync.dma_start(
                        t_sb[:],
                        qT_d[ko * P : (ko + 1) * P, sbk * SBW : (sbk + 1) * SBW],
                    )
                    tiles.append(t_sb)
                qtiles[sbk] = tiles

            def emit_q_chain(m, sbk):
                if sbk not in qtiles:
                    emit_q_tiles(sbk)
                with nc.named_scope("proj_q"):
                    ps = ps_mm.tile([P, SBW], F32, tag="ps", name="ps_q")
                    for ko in range(KO):
                        nc.tensor.matmul(
                            ps[:],
                            wq_sb[:, ko, m * P : (m + 1) * P],
                            qtiles[sbk][ko][:],
                            start=(ko == 0),
                            stop=(ko == KO - 1),
                        )
                    nc.vector.tensor_scalar_add(
                        QpT[:, m, sbk * SBW : (sbk + 1) * SBW],
                        ps[:],
                        bq_sb[:, m : m + 1],
                    )

            # ---- output projection emitter (two n-chains at a time) --------
            def emit_out_proj(sb, otiles, ns):
                with nc.named_scope("proj_out"):
                    for n in ns:
                        psY = ps_mm.tile([P, SBW], F32, tag="ps", name="ps_y")
                        for hp in range(MT):
                            nc.tensor.matmul(
                                psY[:],
                                wo_sb[:, hp, n * P : (n + 1) * P],
                                otiles[hp][:],
                                start=(hp == 0),
                                stop=(hp == MT - 1),
                            )
                        y_sb = yp.tile([P, SBW], F32, tag="y")
                        nc.vector.tensor_copy(y_sb[:], psY[:])
                        nc.sync.dma_start(
                            yT_d[n * P : (n + 1) * P, sb * SBW : (sb + 1) * SBW],
                            y_sb[:],
                        )

            # ---- attention: flat (hp, t) pipeline per s-block ---------------
            def emit_attn_sb(sb, otiles, rate):
                steps = [(hp, t) for hp in range(MT) for t in range(NT)]
                psO = {}
                psS = {}

                def s_mm(hp, t):
                    psT = ps_pair.tile([P, 2 * SBW], F32, tag="psT", name="psT")
                    psS[(hp, t)] = psT
                    for u in range(2):
                        nc.tensor.matmul(
                            psT[:, u * SBW : (u + 1) * SBW],
                            KpT[u * PD : (u + 1) * PD, hp, t * P : (t + 1) * P],
                            QpT[
                                u * PD : (u + 1) * PD,
                                hp,
                                sb * SBW : (sb + 1) * SBW,
                            ],
                            start=True,
                            stop=True,
                            tile_position=(u * PD, 0),
                        )

                with nc.named_scope("attn"):
                    s_mm(*steps[0])
                    for i, (hp, t) in enumerate(steps):
                        pull(rate)
                        if t == 0:
                            psO[hp] = [
                                ps_o.tile([VW, SBW], F32, tag="psO", name=f"psO{u}")
                                for u in range(2)
                            ]
                        if i + 1 < len(steps):
                            s_mm(*steps[i + 1])
                        psT = psS.pop((hp, t))
                        e = expp.tile([P, 2 * SBW], CDT, tag="e")
                        nc.scalar.activation(
                            e[:], psT[:], EXP, scale=1.0 / np.sqrt(PD)
                        )
                        for u in range(2):
                            h = 2 * hp + u
                            nc.tensor.matmul(
                                psO[hp][u][:],
                                Vp[:, t, h * VW : (h + 1) * VW],
                                e[:, u * SBW : (u + 1) * SBW],
                                start=(t == 0),
                                stop=(t == NT - 1),
                            )
                        if t == NT - 1:
                            emit_norm(hp, psO.pop(hp), otiles)

            def emit_norm(hp, psO, otiles):
                with nc.named_scope("norm"):
                    oraw = orp.tile([P, SBW], F32, tag="oraw")
                    nc.vector.tensor_copy(oraw[0:PD, :], psO[0][0:PD, :])
                    nc.vector.tensor_copy(oraw[PD:P, :], psO[1][0:PD, :])
                    den = rcpp.tile([1, 2 * SBW], F32, tag="den")
                    nc.vector.tensor_copy(den[0:1, 0:SBW], psO[0][PD : PD + 1, :])
                    nc.vector.tensor_copy(
                        den[0:1, SBW : 2 * SBW], psO[1][PD : PD + 1, :]
                    )
                    bcd = bcp.tile([P, 2 * SBW], F32, tag="bcd")
                    nc.gpsimd.partition_broadcast(bcd[:], den[:])
                    bcf = bcp.tile([P, 2 * SBW], F32, tag="bc")
                    nc.vector.reciprocal_approx_fast(bcf[:], bcd[:])
                    o_t = op.tile([P, SBW], CDT, tag="o")
                    nc.vector.tensor_tensor(
                        o_t[0:PD, :], oraw[0:PD, :], bcf[0:PD, 0:SBW], MUL
                    )
                    nc.vector.tensor_tensor(
                        o_t[PD:P, :], oraw[PD:P, :], bcf[PD:P, SBW : 2 * SBW], MUL
                    )
                    otiles.append(o_t)

            # ---- schedule ---------------------------------------------------
            # Work-item queue: individual projection matmuls dribbled between
            # attention t-steps so the PE never idles while ACT (exp) runs.
            work_q = []   # urgent: K/Q projection chains (deadline-critical)
            lazy_q = []   # output-projection chains (no deadline)

            def push_k_m(m):
                for sbk in range(NSB):
                    chain = {}

                    def mk(ko, m=m, sbk=sbk, chain=chain):
                        if "ps" not in chain:
                            chain["ps"] = ps_mm.tile(
                                [P, SBW], F32, tag="ps", name="ps_k"
                            )
                        ps = chain["ps"]
                        nc.tensor.matmul(
                            ps[:],
                            wk_sb[:, ko, m * P : (m + 1) * P],
                            ktile[(ko, sbk)][:],
                            start=(ko == 0),
                            stop=(ko == KO - 1),
                        )
                        if ko == KO - 1:
                            nc.vector.tensor_scalar_add(
                                KpT[:, m, sbk * SBW : (sbk + 1) * SBW],
                                ps[:],
                                bk_sb[:, m : m + 1],
                            )
                    for ko in range(KO):
                        work_q.append(mk.__get__ and (lambda ko=ko, mk=mk: mk(ko)))

            def push_q_chain(m, sbk):
                chain = {}

                def mk(ko, m=m, sbk=sbk, chain=chain):
                    if sbk not in qtiles:
                        emit_q_tiles(sbk)
                    if "ps" not in chain:
                        chain["ps"] = ps_mm.tile([P, SBW], F32, tag="ps", name="ps_q")
                    ps = chain["ps"]
                    nc.tensor.matmul(
                        ps[:],
                        wq_sb[:, ko, m * P : (m + 1) * P],
                        qtiles[sbk][ko][:],
                        start=(ko == 0),
                        stop=(ko == KO - 1),
                    )
                    if ko == KO - 1:
                        nc.vector.tensor_scalar_add(
                            QpT[:, m, sbk * SBW : (sbk + 1) * SBW],
                            ps[:],
                            bq_sb[:, m : m + 1],
                        )
                for ko in range(KO):
                    work_q.append(lambda ko=ko, mk=mk: mk(ko))

            def push_y_chains(sb, otiles):
                for n in range(KO):
                    chain = {}

                    def mk(hp, n=n, sb=sb, otiles=otiles, chain=chain):
                        if "ps" not in chain:
                            chain["ps"] = ps_mm.tile(
                                [P, SBW], F32, tag="ps", name="ps_y"
                            )
                        psY = chain["ps"]
                        nc.tensor.matmul(
                            psY[:],
                            wo_sb[:, hp, n * P : (n + 1) * P],
                            otiles[hp][:],
                            start=(hp == 0),
                            stop=(hp == MT - 1),
                        )
                        if hp == MT - 1:
                            y_sb = yp.tile([P, SBW], F32, tag="y")
                            nc.vector.tensor_copy(y_sb[:], psY[:])
                            nc.sync.dma_start(
                                yT_d[n * P : (n + 1) * P, sb * SBW : (sb + 1) * SBW],
                                y_sb[:],
                            )
                    for hp in range(MT):
                        lazy_q.append(lambda hp=hp, mk=mk: mk(hp))

            def pull(n):
                for _ in range(n):
                    if work_q:
                        work_q.pop(0)()
                    elif lazy_q:
                        lazy_q.pop(0)()

            # prelude: K m0 + Q(0,0) emitted directly; everything else queued
            emit_k_m(0)
            emit_q_chain(0, 0)
            # need-by order for the sb=0 block: K m1, Q(0,1), K m2, ...
            push_k_m(1)
            push_q_chain(1, 0)
            push_k_m(2)
            push_q_chain(2, 0)
            push_k_m(3)
            push_q_chain(3, 0)

            otiles_by_sb = {sb: [] for sb in range(NSB)}
            for sb in range(NSB):
                # queue Q chains for the NEXT s-block (need-by: its start)
                if sb + 1 < NSB:
                    for m in range(MT):
                        push_q_chain(m, sb + 1)
                rate = 3 if sb == 0 else 2
                emit_attn_sb(sb, otiles_by_sb[sb], rate)
                push_y_chains(sb, otiles_by_sb[sb])
            # drain whatever is left (final output projections)
            pull(len(work_q) + len(lazy_q))

    nc.compile()
    return nc


def _get_nc():
    global _NC
    if _NC is None:
        _install_ntff_shim()
        _NC = _build()
    return _NC


def make_in_maps(q, k, v, Wq, bq, Wk, bk, Wv, bv, Wo):
    """Shard + lay out the full inputs into the 8 per-core input maps."""
    in_maps = []
    for c in range(NCORES):
        b, j = divmod(c, 2)
        d0 = j * DP
        in_maps.append(
            {
                "qT": np.ascontiguousarray(q[b].T).astype(BF16),
                "kT": np.ascontiguousarray(k[b].T).astype(BF16),
                "vT": np.ascontiguousarray(v[b].T).astype(BF16),
                "wq": np.ascontiguousarray(Wq[:, d0 : d0 + DP]).astype(BF16),
                "wk": np.ascontiguousarray(Wk[:, d0 : d0 + DP]).astype(BF16),
                "wv": np.ascontiguousarray(Wv[:, d0 : d0 + DP]).astype(BF16),
                "wo": np.ascontiguousarray(Wo[d0 : d0 + DP, :]).astype(BF16),
                "bq": np.ascontiguousarray(
                    bq[d0 : d0 + DP].reshape(MT, P).T
                ).astype(np.float32),
                "bk": np.ascontiguousarray(
                    bk[d0 : d0 + DP].reshape(MT, P).T
                ).astype(np.float32),
                "bv": bv[d0 : d0 + DP].reshape(1, DP).astype(np.float32),
            }
        )
    return in_maps


def kernel(q, k, v, Wq, bq, Wk, bk, Wv, bv, Wo, bo, use_causal_mask=1):
    from concourse.bass_utils import run_bass_kernel_spmd

    q = np.asarray(q, np.float32)
    k = np.asarray(k, np.float32)
    v = np.asarray(v, np.float32)
    Wq = np.asarray(Wq, np.float32)
    Wk = np.asarray(Wk, np.float32)
    Wv = np.asarray(Wv, np.float32)
    Wo = np.asarray(Wo, np.float32)
    bq = np.asarray(bq, np.float32)
    bk = np.asarray(bk, np.float32)
    bv = np.asarray(bv, np.float32)
    bo = np.asarray(bo, np.float32)

    nc = _get_nc()
    in_maps = make_in_maps(q, k, v, Wq, bq, Wk, bk, Wv, bv, Wo)
    trace = bool(os.environ.get("KERNEL_TRACE"))
    res = run_bass_kernel_spmd(
        nc, in_maps, core_ids=list(range(NCORES)), trace=trace
    )
    LAST_RUN.clear()
    LAST_RUN.update(
        exec_time_ns=res.exec_time_ns,
        mean_exec_time_ns=res.mean_exec_time_ns,
        trace=(res.instructions_and_trace or (None, None))[1],
        per_core_scope_times=res.per_core_scope_times,
    )

    y = np.empty((B, S, D), np.float32)
    for b in range(B):
        acc = res.results[2 * b]["yT"] + res.results[2 * b + 1]["yT"]
        y[b] = acc.T + bo
    return y


# revision 26
# speedup vs baseline: 1.0323x; 1.0323x over previous
"""Multi-head attention (B=4, S=2048, D=1024, H=16) on 8 TRN2 NeuronCores.

Sharding: 2D — batch 4-way x head-group 2-way. Core c handles batch b = c//2
and head group j = c%2 (8 heads, model-dim slice j*512:(j+1)*512 of the QKV
projections / rows j*512:(j+1)*512 of Wo). Each core computes a partial
[S, D] output (row-sharded Wo => partial sums); the host adds the two
partials per batch and the output bias.

Device-side layout notes:
 - Activations are kept transposed ([feature, seq]) so every matmul has its
   contraction dim on partitions. The host pre-transposes q/k/v (free) and
   pre-casts to bf16 (compute dtype; PSUM accumulation is fp32).
 - Attention uses the S^T layout: S^T[t, s] tiles come straight from
   lhsT=K^T, rhs=Q^T matmuls (two heads row-packed in the PE array);
   softmax-exp runs on ACT (scale=1/8 fused, one op per head pair);
   V is projected in natural [t, d'] layout with a ones-column appended per
   head so the PV matmul also produces the softmax denominators (row 64).
 - The attention phase is ACT(exp)-bound, so the Q/K projection and output
   projection matmul chains are interleaved between attention iterations to
   fill the idle PE slots.
 - The reference's "causal mask" adds log(1 + 1e-6) ~ 1e-6 to allowed
   logits, a ~1e-6 relative perturbation of the output - far below bf16
   noise - so it is deliberately not applied.
"""

import os
import sys
import types

sys.path.insert(0, "/opt/trn_rl_repo")

import numpy as np
import ml_dtypes

B, S, D, H = 4, 2048, 1024, 16
PD = D // H          # 64 head dim
P = 128              # partitions
NCORES = 8
DP = 512             # d' (feature) slice per core = 8 heads
KO = D // P          # 8 contraction tiles for QKV projections
MT = DP // P         # 4 feature tiles (= head pairs)
NT = S // P          # 16 key/time tiles
NSB = 4              # s blocks
SBW = S // NSB       # 512 block width
HL = 8               # local heads
VW = 65              # V columns per head incl. ones column

BF16 = ml_dtypes.bfloat16

_NC = None
LAST_RUN = {}


def _install_ntff_shim():
    """bass_utils' axon trace path imports antenv.axon_hooks, which this
    image lacks; register the hook module manually so BASS_TRACE works."""
    if "antenv.axon_hooks" in sys.modules:
        return
    try:
        import trn_agent_boot.trn_boot as tb

        hook = tb._ntff_profile_via_ctypes("/opt/axon/libaxon_pjrt.so")
    except Exception:
        hook = None
    m = types.ModuleType("antenv.axon_hooks")
    m.get_axon_ntff_profile_hook = lambda: hook
    m.set_axon_ntff_profile_hook = lambda h: None
    sys.modules["antenv.axon_hooks"] = m


def _build():
    import concourse.tile as tile
    from concourse import bacc, mybir

    CDT = mybir.dt.bfloat16
    F32 = mybir.dt.float32
    EXP = mybir.ActivationFunctionType.Exp
    ADD = mybir.AluOpType.add
    MUL = mybir.AluOpType.mult

    nc = bacc.Bacc("TRN2", target_bir_lowering=False, debug=False)

    qT_d = nc.dram_tensor("qT", [D, S], CDT, kind="ExternalInput").ap()
    kT_d = nc.dram_tensor("kT", [D, S], CDT, kind="ExternalInput").ap()
    vT_d = nc.dram_tensor("vT", [D, S], CDT, kind="ExternalInput").ap()
    wq_d = nc.dram_tensor("wq", [D, DP], CDT, kind="ExternalInput").ap()
    wk_d = nc.dram_tensor("wk", [D, DP], CDT, kind="ExternalInput").ap()
    wv_d = nc.dram_tensor("wv", [D, DP], CDT, kind="ExternalInput").ap()
    wo_d = nc.dram_tensor("wo", [DP, D], CDT, kind="ExternalInput").ap()
    bq_d = nc.dram_tensor("bq", [P, MT], F32, kind="ExternalInput").ap()
    bk_d = nc.dram_tensor("bk", [P, MT], F32, kind="ExternalInput").ap()
    bv_d = nc.dram_tensor("bv", [1, DP], F32, kind="ExternalInput").ap()
    yT_d = nc.dram_tensor("yT", [D, S], F32, kind="ExternalOutput").ap()

    with tile.TileContext(nc) as tc:
        import contextlib

        with contextlib.ExitStack() as ctx:
            wp = ctx.enter_context(tc.tile_pool(name="weights", bufs=1))
            biasp = ctx.enter_context(tc.tile_pool(name="bias", bufs=1))
            inp = ctx.enter_context(tc.tile_pool(name="inp", bufs=16))
            kinp = ctx.enter_context(tc.tile_pool(name="kinp", bufs=32))
            qsp = ctx.enter_context(tc.tile_pool(name="qsmall", bufs=16))
            actp = ctx.enter_context(tc.tile_pool(name="acts", bufs=1))
            expp = ctx.enter_context(tc.tile_pool(name="exps", bufs=4))
            op = ctx.enter_context(tc.tile_pool(name="otile", bufs=8))
            orp = ctx.enter_context(tc.tile_pool(name="oraw", bufs=2))
            rcpp = ctx.enter_context(tc.tile_pool(name="rcp", bufs=3))
            bcp = ctx.enter_context(tc.tile_pool(name="bcast", bufs=1))
            yp = ctx.enter_context(tc.tile_pool(name="ystage", bufs=2))
            # PSUM budget (8 banks): S^T pair tiles 2x2 + proj/out 2 + psO 2
            ps_pair = ctx.enter_context(
                tc.tile_pool(name="ps_pair", bufs=2, space="PSUM")
            )
            ps_mm = ctx.enter_context(tc.tile_pool(name="ps_mm", bufs=2, space="PSUM"))
            ps_o = ctx.enter_context(tc.tile_pool(name="ps_o", bufs=2, space="PSUM"))

            # ---- biases + weights (order matters: V first, K, then Q/out) --
            bv_sb = biasp.tile([P, DP], F32, tag="bv")
            nc.sync.dma_start(bv_sb[:], bv_d[0:1, :].to_broadcast((P, DP)))
            bq_sb = biasp.tile([P, MT], F32, tag="bq")
            nc.sync.dma_start(bq_sb[:], bq_d[:])
            bk_sb = biasp.tile([P, MT], F32, tag="bk")
            nc.sync.dma_start(bk_sb[:], bk_d[:])
            wv_sb = wp.tile([P, KO, DP], CDT, tag="wv")
            nc.sync.dma_start(wv_sb[:], wv_d.rearrange("(a p) m -> p a m", p=P))

            # vT in halves so the V projection can start after ~half the DMA;
            # kT in s-block quarters so K chains start per-block.
            vtiles = []
            for ko in range(KO):
                halves = []
                for h in range(2):
                    t_sb = inp.tile([P, S // 2], CDT, tag="vT", name=f"v{ko}_{h}")
                    nc.sync.dma_start(
                        t_sb[:],
                        vT_d[ko * P : (ko + 1) * P, h * (S // 2) : (h + 1) * (S // 2)],
                    )
                    halves.append(t_sb)
                vtiles.append(halves)
            wq_sb = wp.tile([P, KO, DP], CDT, tag="wq")
            nc.sync.dma_start(wq_sb[:], wq_d.rearrange("(a p) m -> p a m", p=P))

            qtiles = {}  # sbk -> list of 8 [P, SBW] tiles

            def emit_q_tiles(sbk):
                tiles = []
                for ko in range(KO):
                    t_sb = qsp.tile([P, SBW], CDT, tag="qT", name=f"q{ko}")
                    nc.sync.dma_start(
                        t_sb[:],
                        qT_d[ko * P : (ko + 1) * P, sbk * SBW : (sbk + 1) * SBW],
                    )
                    tiles.append(t_sb)
                qtiles[sbk] = tiles

            emit_q_tiles(0)
            wk_sb = wp.tile([P, KO, DP], CDT, tag="wk")
            nc.sync.dma_start(wk_sb[:], wk_d.rearrange("(a p) m -> p a m", p=P))
            ktiles = []
            for sbk in range(NSB):
                for ko in range(KO):
                    t_sb = kinp.tile([P, SBW], CDT, tag="kT", name=f"k{ko}_{sbk}")
                    nc.sync.dma_start(
                        t_sb[:],
                        kT_d[ko * P : (ko + 1) * P, sbk * SBW : (sbk + 1) * SBW],
                    )
                    ktiles.append((ko, sbk, t_sb))
            ktile = {(ko, sbk): t for ko, sbk, t in ktiles}
            wo_sb = wp.tile([P, MT, D], CDT, tag="wo")
            nc.sync.dma_start(wo_sb[:], wo_d.rearrange("(a p) n -> p a n", p=P))

            # ---- V projection (natural layout [t, d'], ones col per head) --
            Vp = actp.tile([P, NT, HL * VW], CDT, tag="Vp")
            with nc.named_scope("proj_v"):
                nc.vector.memset(
                    Vp[:].rearrange("p t (h c) -> p t h c", c=VW)[:, :, :, PD : PD + 1],
                    1.0,
                )
                for t in range(NT):
                    ps = ps_mm.tile([P, DP], F32, tag="ps", name="ps_v")
                    for ko in range(KO):
                        half = t // (NT // 2)
                        tt = t % (NT // 2)
                        nc.tensor.matmul(
                            ps[:],
                            vtiles[ko][half][:, tt * P : (tt + 1) * P],
                            wv_sb[:, ko, :],
                            start=(ko == 0),
                            stop=(ko == KO - 1),
                        )
                    nc.vector.tensor_tensor(
                        Vp[:, t, :].rearrange("p (h c) -> p h c", c=VW)[:, :, 0:PD],
                        ps[:].rearrange("p (h c) -> p h c", c=PD),
                        bv_sb[:].rearrange("p (h c) -> p h c", c=PD),
                        ADD,
                    )

            # ---- K / Q projection chain emitters ---------------------------
            KpT = actp.tile([P, MT, S], CDT, tag="KpT")
            QpT = actp.tile([P, MT, S], CDT, tag="QpT")

            def emit_k_m(m):
                with nc.named_scope("proj_k"):
                    for sbk in range(NSB):
                        ps = ps_mm.tile([P, SBW], F32, tag="ps", name="ps_k")
                        for ko in range(KO):
                            nc.tensor.matmul(
                                ps[:],
                                wk_sb[:, ko, m * P : (m + 1) * P],
                                ktile[(ko, sbk)][:],
                                start=(ko == 0),
                                stop=(ko == KO - 1),
                            )
                        nc.vector.tensor_scalar_add(
                            KpT[:, m, sbk * SBW : (sbk + 1) * SBW],
                            ps[:],
                            bk_sb[:, m : m + 1],
                        )

            def emit_q_chain(m, sbk):
                if sbk not in qtiles:
                    emit_q_tiles(sbk)
                with nc.named_scope("proj_q"):
                    ps = ps_mm.tile([P, SBW], F32, tag="ps", name="ps_q")
                    for ko in range(KO):
                        nc.tensor.matmul(
                            ps[:],
                            wq_sb[:, ko, m * P : (m + 1) * P],
                            qtiles[sbk][ko][:],
                            start=(ko == 0),
                            stop=(ko == KO - 1),
                        )
                    nc.vector.tensor_scalar_add(
                        QpT[:, m, sbk * SBW : (sbk + 1) * SBW],
                        ps[:],
                        bq_sb[:, m : m + 1],
                    )

            # ---- output projection emitter (two n-chains at a time) --------
            def emit_out_proj(sb, otiles, ns):
                with nc.named_scope("proj_out"):
                    for n in ns:
                        psY = ps_mm.tile([P, SBW], F32, tag="ps", name="ps_y")
                        for hp in range(MT):
                            nc.tensor.matmul(
                                psY[:],
                                wo_sb[:, hp, n * P : (n + 1) * P],
                                otiles[hp][:],
                                start=(hp == 0),
                                stop=(hp == MT - 1),
                            )
                        y_sb = yp.tile([P, SBW], F32, tag="y")
                        nc.vector.tensor_copy(y_sb[:], psY[:])
                        nc.sync.dma_start(
                            yT_d[n * P : (n + 1) * P, sb * SBW : (sb + 1) * SBW],
                            y_sb[:],
                        )

            # ---- attention: flat (hp, t) pipeline per s-block ---------------
            def emit_attn_sb(sb, otiles, rate):
                steps = [(hp, t) for hp in range(MT) for t in range(NT)]
                psO = {}
                psS = {}

                def s_mm(hp, t):
                    psT = ps_pair.tile([P, 2 * SBW], F32, tag="psT", name="psT")
                    psS[(hp, t)] = psT
                    for u in range(2):
                        nc.tensor.matmul(
                            psT[:, u * SBW : (u + 1) * SBW],
                            KpT[u * PD : (u + 1) * PD, hp, t * P : (t + 1) * P],
                            QpT[
                                u * PD : (u + 1) * PD,
                                hp,
                                sb * SBW : (sb + 1) * SBW,
                            ],
                            start=True,
                            stop=True,
                            tile_position=(u * PD, 0),
                        )

                with nc.named_scope("attn"):
                    s_mm(*steps[0])
                    for i, (hp, t) in enumerate(steps):
                        pull(rate)
                        if t == 0:
                            psO[hp] = [
                                ps_o.tile([VW, SBW], F32, tag="psO", name=f"psO{u}")
                                for u in range(2)
                            ]
                        if i + 1 < len(steps):
                            s_mm(*steps[i + 1])
                        psT = psS.pop((hp, t))
                        e = expp.tile([P, 2 * SBW], CDT, tag="e")
                        nc.scalar.activation(
                            e[:], psT[:], EXP, scale=1.0 / np.sqrt(PD)
                        )
                        for u in range(2):
                            h = 2 * hp + u
                            nc.tensor.matmul(
                                psO[hp][u][:],
                                Vp[:, t, h * VW : (h + 1) * VW],
                                e[:, u * SBW : (u + 1) * SBW],
                                start=(t == 0),
                                stop=(t == NT - 1),
                            )
                        if t == NT - 1:
                            emit_norm(hp, psO.pop(hp), otiles)

            def emit_norm(hp, psO, otiles):
                with nc.named_scope("norm"):
                    oraw = orp.tile([P, SBW], F32, tag="oraw")
                    nc.vector.tensor_copy(oraw[0:PD, :], psO[0][0:PD, :])
                    nc.vector.tensor_copy(oraw[PD:P, :], psO[1][0:PD, :])
                    den = rcpp.tile([1, 2 * SBW], F32, tag="den")
                    nc.vector.tensor_copy(den[0:1, 0:SBW], psO[0][PD : PD + 1, :])
                    nc.vector.tensor_copy(
                        den[0:1, SBW : 2 * SBW], psO[1][PD : PD + 1, :]
                    )
                    bcd = bcp.tile([P, 2 * SBW], F32, tag="bcd")
                    nc.gpsimd.partition_broadcast(bcd[:], den[:])
                    bcf = bcp.tile([P, 2 * SBW], F32, tag="bc")
                    nc.vector.reciprocal_approx_fast(bcf[:], bcd[:])
                    o_t = op.tile([P, SBW], CDT, tag="o")
                    nc.vector.tensor_tensor(
                        o_t[0:PD, :], oraw[0:PD, :], bcf[0:PD, 0:SBW], MUL
                    )
                    nc.vector.tensor_tensor(
                        o_t[PD:P, :], oraw[PD:P, :], bcf[PD:P, SBW : 2 * SBW], MUL
                    )
                    otiles.append(o_t)

            # ---- schedule ---------------------------------------------------
            # Work-item queue: individual projection matmuls dribbled between
            # attention t-steps so the PE never idles while ACT (exp) runs.
            work_q = []   # urgent: K/Q projection chains (deadline-critical)
            lazy_q = []   # output-projection chains (no deadline)

            def push_k_m(m):
                for sbk in range(NSB):
                    chain = {}

                    def mk(ko, m=m, sbk=sbk, chain=chain):
                        if "ps" not in chain:
                            chain["ps"] = ps_mm.tile(
                                [P, SBW], F32, tag="ps", name="ps_k"
                            )
                        ps = chain["ps"]
                        nc.tensor.matmul(
                            ps[:],
                            wk_sb[:, ko, m * P : (m + 1) * P],
                            ktile[(ko, sbk)][:],
                            start=(ko == 0),
                            stop=(ko == KO - 1),
                        )
                        if ko == KO - 1:
                            nc.vector.tensor_scalar_add(
                                KpT[:, m, sbk * SBW : (sbk + 1) * SBW],
                                ps[:],
                                bk_sb[:, m : m + 1],
                            )
                    for ko in range(KO):
                        work_q.append(mk.__get__ and (lambda ko=ko, mk=mk: mk(ko)))

            def push_q_chain(m, sbk):
                chain = {}

                def mk(ko, m=m, sbk=sbk, chain=chain):
                    if sbk not in qtiles:
                        emit_q_tiles(sbk)
                    if "ps" not in chain:
                        chain["ps"] = ps_mm.tile([P, SBW], F32, tag="ps", name="ps_q")
                    ps = chain["ps"]
                    nc.tensor.matmul(
                        ps[:],
                        wq_sb[:, ko, m * P : (m + 1) * P],
                        qtiles[sbk][ko][:],
                        start=(ko == 0),
                        stop=(ko == KO - 1),
                    )
                    if ko == KO - 1:
                        nc.vector.tensor_scalar_add(
                            QpT[:, m, sbk * SBW : (sbk + 1) * SBW],
                            ps[:],
                            bq_sb[:, m : m + 1],
                        )
                for ko in range(KO):
                    work_q.append(lambda ko=ko, mk=mk: mk(ko))

            def push_y_chains(sb, otiles):
                for n in range(KO):
                    chain = {}

                    def mk(hp, n=n, sb=sb, otiles=otiles, chain=chain):
                        if "ps" not in chain:
                            chain["ps"] = ps_mm.tile(
                                [P, SBW], F32, tag="ps", name="ps_y"
                            )
                        psY = chain["ps"]
                        nc.tensor.matmul(
                            psY[:],
                            wo_sb[:, hp, n * P : (n + 1) * P],
                            otiles[hp][:],
                            start=(hp == 0),
                            stop=(hp == MT - 1),
                        )
                        if hp == MT - 1:
                            y_sb = yp.tile([P, SBW], F32, tag="y")
                            nc.vector.tensor_copy(y_sb[:], psY[:])
                            nc.sync.dma_start(
                                yT_d[n * P : (n + 1) * P, sb * SBW : (sb + 1) * SBW],
                                y_sb[:],
                            )
                    for hp in range(MT):
                        lazy_q.append(lambda hp=hp, mk=mk: mk(hp))

            def pull(n):
                for _ in range(n):
                    if work_q:
                        work_q.pop(0)()
                    elif lazy_q:
                        lazy_q.pop(0)()

            # prelude: K m0 + Q(0,0) emitted directly; everything else queued
            emit_k_m(0)
            emit_q_chain(0, 0)
            # need-by order for the sb=0 block: K m1, Q(0,1), K m2, ...
            push_k_m(1)
            push_q_chain(1, 0)
            push_k_m(2)
            push_q_chain(2, 0)
            push_k_m(3)
            push_q_chain(3, 0)

            otiles_by_sb = {sb: [] for sb in range(NSB)}
            for sb in range(NSB):
                # queue Q chains for the NEXT s-block (need-by: its start)
                if sb + 1 < NSB:
                    for m in range(MT):
                        push_q_chain(m, sb + 1)
                rate = 3 if sb == 0 else 1
                emit_attn_sb(sb, otiles_by_sb[sb], rate)
                push_y_chains(sb, otiles_by_sb[sb])
            # drain whatever is left (final output projections)
            pull(len(work_q) + len(lazy_q))

    nc.compile()
    return nc


def _get_nc():
    global _NC
    if _NC is None:
        _install_ntff_shim()
        _NC = _build()
    return _NC


def make_in_maps(q, k, v, Wq, bq, Wk, bk, Wv, bv, Wo):
    """Shard + lay out the full inputs into the 8 per-core input maps."""
    in_maps = []
    for c in range(NCORES):
        b, j = divmod(c, 2)
        d0 = j * DP
        in_maps.append(
            {
                "qT": np.ascontiguousarray(q[b].T).astype(BF16),
                "kT": np.ascontiguousarray(k[b].T).astype(BF16),
                "vT": np.ascontiguousarray(v[b].T).astype(BF16),
                "wq": np.ascontiguousarray(Wq[:, d0 : d0 + DP]).astype(BF16),
                "wk": np.ascontiguousarray(Wk[:, d0 : d0 + DP]).astype(BF16),
                "wv": np.ascontiguousarray(Wv[:, d0 : d0 + DP]).astype(BF16),
                "wo": np.ascontiguousarray(Wo[d0 : d0 + DP, :]).astype(BF16),
                "bq": np.ascontiguousarray(
                    bq[d0 : d0 + DP].reshape(MT, P).T
                ).astype(np.float32),
                "bk": np.ascontiguousarray(
                    bk[d0 : d0 + DP].reshape(MT, P).T
                ).astype(np.float32),
                "bv": bv[d0 : d0 + DP].reshape(1, DP).astype(np.float32),
            }
        )
    return in_maps


def kernel(q, k, v, Wq, bq, Wk, bk, Wv, bv, Wo, bo, use_causal_mask=1):
    from concourse.bass_utils import run_bass_kernel_spmd

    q = np.asarray(q, np.float32)
    k = np.asarray(k, np.float32)
    v = np.asarray(v, np.float32)
    Wq = np.asarray(Wq, np.float32)
    Wk = np.asarray(Wk, np.float32)
    Wv = np.asarray(Wv, np.float32)
    Wo = np.asarray(Wo, np.float32)
    bq = np.asarray(bq, np.float32)
    bk = np.asarray(bk, np.float32)
    bv = np.asarray(bv, np.float32)
    bo = np.asarray(bo, np.float32)

    nc = _get_nc()
    in_maps = make_in_maps(q, k, v, Wq, bq, Wk, bk, Wv, bv, Wo)
    trace = bool(os.environ.get("KERNEL_TRACE"))
    res = run_bass_kernel_spmd(
        nc, in_maps, core_ids=list(range(NCORES)), trace=trace
    )
    LAST_RUN.clear()
    LAST_RUN.update(
        exec_time_ns=res.exec_time_ns,
        mean_exec_time_ns=res.mean_exec_time_ns,
        trace=(res.instructions_and_trace or (None, None))[1],
        per_core_scope_times=res.per_core_scope_times,
    )

    y = np.empty((B, S, D), np.float32)
    for b in range(B):
        acc = res.results[2 * b]["yT"] + res.results[2 * b + 1]["yT"]
        y[b] = acc.T + bo
    return y
